# revision 6
# baseline (speedup 1.0000x reference)
"""Kinetic-optimal discrete Euler solver — Trainium2 Bass kernel.

Computes row[n, :] = u_t[n, x_t[n], :] of the kinetic-optimal rate matrix
without materializing the [N, V, V] tensor the reference builds.

Math (per token n, with i = x_t[n], c = x_1[n], s = source_p):
    q[j]    = onehot_c[j] - s[j]
    p_t[j]  = s[j] + k*q[j]                  ( = (1-k)s + k*onehot_c )
    p_dot[j]= dk*q[j]
    a       = p_t[i] = (1-k)*s[i] + k*[i==c]
    b       = p_dot[i] = dk*([i==c] - s[i])
    num[j]  = relu(a*p_dot[j] - b*p_t[j])
    u[j]    = num[j] / (p_t[j] + eps)
    row[j]  = u[j] - [j==i] * sum_j u[j]
(The diagonal term u[i] is exactly 0 pre-subtraction, so the row only
needs its own row-sum — O(N*V) total work.)

Sharding: tokens N=512 split 64-per-core across 8 NeuronCores (pure data
parallel); source_p / scheduler scalars replicated.
"""

import numpy as np
from contextlib import ExitStack

N = 512
V = 512
NCORES = 8
NT = N // NCORES  # tokens per core
EPS = 1e-8

_CACHE = {}


def _build_bass():
    import concourse.bass as bass
    import concourse.mybir as mybir
    from concourse import bacc
    from concourse import tile

    fp32 = mybir.dt.float32
    Alu = mybir.AluOpType

    nc = bacc.Bacc("TRN2", target_bir_lowering=False, debug=False)

    # DRAM I/O (host pre-replicates broadcasts; indices arrive as f32)
    s_d = nc.dram_tensor("s_b", [NT, V], fp32, kind="ExternalInput")
    se_d = nc.dram_tensor("se_b", [NT, V], fp32, kind="ExternalInput")
    io_d = nc.dram_tensor("io_b", [NT, V], fp32, kind="ExternalInput")
    xt_d = nc.dram_tensor("xt", [NT, 1], fp32, kind="ExternalInput")
    x1_d = nc.dram_tensor("x1", [NT, 1], fp32, kind="ExternalInput")
    k_d = nc.dram_tensor("kk", [NT, 1], fp32, kind="ExternalInput")
    dk_d = nc.dram_tensor("dk", [NT, 1], fp32, kind="ExternalInput")
    out_d = nc.dram_tensor("out", [NT, V], fp32, kind="ExternalOutput")

    with tile.TileContext(nc) as tc, ExitStack() as ctx:
        pool = ctx.enter_context(tc.tile_pool(name="main", bufs=1))

        def big(tag):
            return pool.tile([NT, V], fp32, name=tag, tag=tag)

        def small(tag):
            return pool.tile([NT, 1], fp32, name=tag, tag=tag)

        s_t, se_t, io_t = big("s_t"), big("se_t"), big("io_t")
        xt_t, x1_t, k_t, dk_t = (
            small("xt_t"),
            small("x1_t"),
            small("k_t"),
            small("dk_t"),
        )

        nc.sync.dma_start(io_t[:], io_d.ap())
        nc.sync.dma_start(s_t[:], s_d.ap())
        nc.sync.dma_start(se_t[:], se_d.ap())
        nc.sync.dma_start(xt_t[:], xt_d.ap())
        nc.sync.dma_start(x1_t[:], x1_d.ap())
        nc.sync.dma_start(k_t[:], k_d.ap())
        nc.sync.dma_start(dk_t[:], dk_d.ap())

        dc, di = big("dc"), big("di")
        nc.vector.tensor_scalar(dc[:], io_t[:], x1_t[:], None, Alu.is_equal)
        nc.vector.tensor_scalar(di[:], io_t[:], xt_t[:], None, Alu.is_equal)

        # s_xt[n] = s[x_t[n]] via one-hot dot
        junk, s_xt = big("junk"), small("s_xt")
        nc.vector.tensor_tensor(junk[:], di[:], s_t[:], Alu.mult)
        nc.vector.tensor_reduce(s_xt[:], junk[:], mybir.AxisListType.X, Alu.add)

        eq = small("eq")
        nc.vector.tensor_tensor(eq[:], xt_t[:], x1_t[:], Alu.is_equal)

        # per-token scalars
        omk, t_as, a_t, b_t, nbe = small("omk"), small("t_as"), small("a_t"), small("b_t"), small("nbe")
        nc.vector.tensor_scalar(omk[:], k_t[:], -1.0, 1.0, Alu.mult, Alu.add)
        nc.vector.tensor_scalar(t_as[:], s_xt[:], omk[:], None, Alu.mult)
        nc.vector.tensor_scalar(a_t[:], eq[:], k_t[:], t_as[:], Alu.mult, Alu.add)
        nc.vector.tensor_scalar(b_t[:], eq[:], s_xt[:], dk_t[:], Alu.subtract, Alu.mult)
        nc.vector.tensor_scalar(nbe[:], b_t[:], -EPS, None, Alu.mult)

        q, kq, pden, rec = big("q"), big("kq"), big("pden"), big("rec")
        nc.vector.tensor_tensor(q[:], dc[:], s_t[:], Alu.subtract)
        nc.vector.tensor_scalar(kq[:], q[:], k_t[:], None, Alu.mult)
        nc.vector.tensor_tensor(pden[:], kq[:], se_t[:], Alu.add)
        nc.vector.reciprocal(rec[:], pden[:])

        t1, t2, pre, num = big("t1"), big("t2"), big("pre"), big("num")
        nc.vector.tensor_scalar(t1[:], q[:], a_t[:], dk_t[:], Alu.mult, Alu.mult)
        nc.vector.tensor_scalar(t2[:], pden[:], b_t[:], nbe[:], Alu.mult, Alu.add)
        nc.vector.tensor_tensor(pre[:], t1[:], t2[:], Alu.subtract)
        nc.vector.tensor_scalar(num[:], pre[:], 0.0, None, Alu.max)

        u, rowsum = big("u"), small("rowsum")
        nc.vector.tensor_tensor(u[:], num[:], rec[:], Alu.mult)
        nc.vector.tensor_reduce(rowsum[:], u[:], mybir.AxisListType.X, Alu.add)

        t3, row = big("t3"), big("row")
        nc.vector.tensor_scalar(t3[:], di[:], rowsum[:], None, Alu.mult)
        nc.vector.tensor_tensor(row[:], u[:], t3[:], Alu.subtract)

        nc.sync.dma_start(out_d.ap(), row[:])

    nc.compile()
    return nc


def _get_nc():
    if "nc" not in _CACHE:
        _CACHE["nc"] = _build_bass()
    return _CACHE["nc"]


def _in_maps(source_p, k_t, d_k_t, x_t, x_1):
    s = np.asarray(source_p, dtype=np.float32).reshape(V)
    kf = np.float32(np.asarray(k_t).reshape(()))
    dkf = np.float32(np.asarray(d_k_t).reshape(()))
    xt = np.asarray(x_t).astype(np.float32).reshape(N, 1)
    x1 = np.asarray(x_1).astype(np.float32).reshape(N, 1)

    s_b = np.ascontiguousarray(np.broadcast_to(s, (NT, V)))
    se_b = np.ascontiguousarray(np.broadcast_to(s + np.float32(EPS), (NT, V)))
    io_b = np.ascontiguousarray(
        np.broadcast_to(np.arange(V, dtype=np.float32), (NT, V))
    )
    kk = np.full((NT, 1), kf, dtype=np.float32)
    dk = np.full((NT, 1), dkf, dtype=np.float32)

    maps = []
    for c in range(NCORES):
        lo, hi = c * NT, (c + 1) * NT
        maps.append(
            {
                "s_b": s_b,
                "se_b": se_b,
                "io_b": io_b,
                "xt": np.ascontiguousarray(xt[lo:hi]),
                "x1": np.ascontiguousarray(x1[lo:hi]),
                "kk": kk,
                "dk": dk,
            }
        )
    return maps


def kernel(source_p, k_t, d_k_t, x_t, x_1):
    from concourse.bass_utils import run_bass_kernel_spmd

    nc = _get_nc()
    maps = _in_maps(source_p, k_t, d_k_t, x_t, x_1)
    res = run_bass_kernel_spmd(nc, maps, list(range(NCORES)))
    out = np.concatenate([res.results[c]["out"] for c in range(NCORES)], axis=0)
    return out.astype(np.float32)


# revision 8
# speedup vs baseline: 1.3521x; 1.3521x over previous
"""v6: [128,256] packing — 2 partitions per token, halved free dim.

Host gives shifted indices (xt' = xt - 256*parity) so each partition
compares against a plain 0..255 iota for its half of the vocab. Pair
sums (s[x_t] one-hot dot, row sum) go through a tiny K=128 N=1 PE
matmul with a block-pair matrix, broadcasting back to both partitions.

Chain reorder: u = relu(a2*(q*rec) - b*(1 - eps*rec)) using pden*rec==1,
so the per-token scalars (from the pair-summed one-hot dot) are only
needed late in the DVE stream.
"""
import numpy as np
from contextlib import ExitStack

N = 512
V = 512
NCORES = 8
NT = N // NCORES  # 64 tokens/core
P = 2 * NT        # 128 partitions
H = V // 2        # 256 free
EPS = 1e-8


def build_default():
    import concourse.bass as bass
    import concourse.mybir as mybir
    from concourse import bacc
    from concourse import tile

    fp32 = mybir.dt.float32
    Alu = mybir.AluOpType
    Act = mybir.ActivationFunctionType

    nc = bacc.Bacc("TRN2", target_bir_lowering=False, debug=False)

    W = 4 + H + H + P  # packed input width: sm | io2 | s2 | mm
    pk_d = nc.dram_tensor("pk", [P, W], fp32, kind="ExternalInput")
    out_d = nc.dram_tensor("out", [NT, V], fp32, kind="ExternalOutput")

    with tile.TileContext(nc) as tc, ExitStack() as ctx:
        pool = ctx.enter_context(tc.tile_pool(name="main", bufs=1))
        psum = ctx.enter_context(tc.tile_pool(name="ps", bufs=1, space="PSUM"))

        def big(tag, dt=fp32):
            return pool.tile([P, H], dt, name=tag, tag=tag)

        def small(tag, dt=fp32):
            return pool.tile([P, 1], dt, name=tag, tag=tag)

        pk_t = pool.tile([P, W], fp32, name="pk_t")

        # ACT func-table prewarm overlapping the DMA front
        warm = pool.tile([1, 1], fp32, name="warm")
        nc.gpsimd.memset(warm[:], 0.0)
        nc.scalar.activation(warm[:], warm[:], Act.Copy, bias=0.0)

        nc.sync.dma_start(pk_t[:], pk_d.ap())

        xt_c, x1_c = pk_t[:, 0:1], pk_t[:, 1:2]
        k_c, dk_c = pk_t[:, 2:3], pk_t[:, 3:4]
        io_t = pk_t[:, 4 : 4 + H]
        s_t = pk_t[:, 4 + H : 4 + 2 * H]
        m_t = pk_t[:, 4 + 2 * H : 4 + 2 * H + P]

        # s + eps on ACT (off the DVE stream)
        se_t = big("se_t")
        nc.scalar.activation(se_t[:], s_t, Act.Copy, bias=EPS)

        # DVE stream, in emission order
        dc, di, junk = big("dc"), big("di"), big("junk")
        sxth = small("sxth")
        nc.vector.tensor_scalar(dc[:], io_t, x1_c, None, Alu.is_equal)
        nc.vector.tensor_scalar(di[:], io_t, xt_c, None, Alu.is_equal)
        nc.vector.tensor_tensor(junk[:], di[:], s_t, Alu.mult)
        nc.vector.tensor_reduce(sxth[:], junk[:], mybir.AxisListType.X, Alu.add)

        # pair-sum s_xt across the partition pair on PE, copy back to SBUF
        sxt_p = psum.tile([P, 1], fp32, name="sxt_p")
        s_xt = small("s_xt")
        nc.tensor.matmul(sxt_p[:], lhsT=m_t, rhs=sxth[:], start=True, stop=True)
        nc.scalar.activation(s_xt[:], sxt_p[:], Act.Copy, bias=0.0)

        # per-token scalars: omk/t_as on ACT (idle), rest inline on DVE later
        # (gpsimd tensor ops crash the neuronxcc compile — keep Pool DMA-free)
        eq = small("eq")
        omk = small("omk")
        t_as = small("t_as")
        a1 = small("a1")
        b_t = small("b_t")
        nc.scalar.activation(omk[:], k_c, Act.Copy, scale=-1.0, bias=1.0)
        nc.scalar.activation(t_as[:], s_xt[:], Act.Copy, scale=omk[:], bias=0.0)

        # main chain
        q, kq, pden, rec = big("q"), big("kq"), big("pden"), big("rec")
        v_t, g, y, z = big("v_t"), big("g"), big("y"), big("z")
        e, u = big("e"), big("u")
        rsh = small("rsh")
        nc.vector.tensor_tensor(q[:], dc[:], s_t, Alu.subtract)
        nc.vector.tensor_scalar(kq[:], q[:], k_c, None, Alu.mult)
        nc.vector.tensor_tensor(pden[:], kq[:], se_t[:], Alu.add)
        nc.vector.reciprocal(rec[:], pden[:])
        nc.vector.tensor_scalar(v_t[:], rec[:], -EPS, 1.0, Alu.mult, Alu.add)
        nc.vector.tensor_tensor(g[:], q[:], rec[:], Alu.mult)
        nc.vector.tensor_tensor(eq[:], xt_c, x1_c, Alu.is_equal)
        nc.vector.tensor_scalar(a1[:], eq[:], k_c, t_as[:], Alu.mult, Alu.add)
        nc.vector.tensor_scalar(b_t[:], eq[:], s_xt[:], dk_c, Alu.subtract, Alu.mult)
        nc.vector.tensor_scalar(y[:], g[:], a1[:], dk_c, Alu.mult, Alu.mult)
        nc.vector.tensor_scalar(z[:], v_t[:], b_t[:], None, Alu.mult)
        nc.vector.tensor_tensor(e[:], y[:], z[:], Alu.subtract)
        nc.vector.tensor_scalar(u[:], e[:], 0.0, None, Alu.max)
        nc.vector.tensor_reduce(rsh[:], u[:], mybir.AxisListType.X, Alu.add)

        # pair-sum row sums on PE
        rs_p = psum.tile([P, 1], fp32, name="rs_p")
        rowsum = small("rowsum")
        nc.tensor.matmul(rs_p[:], lhsT=m_t, rhs=rsh[:], start=True, stop=True)
        nc.scalar.activation(rowsum[:], rs_p[:], Act.Copy, bias=0.0)

        t3, row = big("t3"), big("row")
        nc.vector.tensor_scalar(t3[:], di[:], rowsum[:], None, Alu.mult)
        nc.vector.tensor_tensor(row[:], u[:], t3[:], Alu.subtract)

        out_ap = out_d.ap().rearrange("a (h b) -> (a h) b", h=2)
        nc.sync.dma_start(out_ap, row[:])

    nc.compile()
    return nc


def in_maps(source_p, k_t, d_k_t, x_t, x_1):
    s = np.asarray(source_p, dtype=np.float32).reshape(V)
    kf = np.float32(np.asarray(k_t).reshape(()))
    dkf = np.float32(np.asarray(d_k_t).reshape(()))
    xt = np.asarray(x_t).reshape(N).astype(np.int64)
    x1 = np.asarray(x_1).reshape(N).astype(np.int64)

    W = 4 + H + H + P
    parity = np.tile(np.array([0, 1], dtype=np.int64), NT)  # per partition

    base = np.empty((P, W), dtype=np.float32)
    base[:, 4 : 4 + H] = np.arange(H, dtype=np.float32)[None, :]
    base[0::2, 4 + H : 4 + 2 * H] = s[:H]
    base[1::2, 4 + H : 4 + 2 * H] = s[H:]
    base[:, 4 + 2 * H :] = np.kron(
        np.eye(NT, dtype=np.float32), np.ones((2, 2), dtype=np.float32)
    )
    base[:, 2] = kf
    base[:, 3] = dkf

    maps = []
    for c in range(NCORES):
        lo, hi = c * NT, (c + 1) * NT
        pk = base.copy()
        pk[:, 0] = (np.repeat(xt[lo:hi], 2) - H * parity).astype(np.float32)
        pk[:, 1] = (np.repeat(x1[lo:hi], 2) - H * parity).astype(np.float32)
        maps.append({"pk": pk})
    return maps


_CACHE = {}


def _get_nc():
    if "nc" not in _CACHE:
        _CACHE["nc"] = build_default()
    return _CACHE["nc"]


def _in_maps(source_p, k_t, d_k_t, x_t, x_1):
    return in_maps(source_p, k_t, d_k_t, x_t, x_1)


def kernel(source_p, k_t, d_k_t, x_t, x_1):
    from concourse.bass_utils import run_bass_kernel_spmd

    nc = _get_nc()
    maps = in_maps(source_p, k_t, d_k_t, x_t, x_1)
    res = run_bass_kernel_spmd(nc, maps, list(range(NCORES)))
    out = np.concatenate([res.results[c]["out"] for c in range(NCORES)], axis=0)
    return out.astype(np.float32)
